# revision 24
# baseline (speedup 1.0000x reference)
"""Trainium2 Bass kernel for nn_PolarOut (segment_reduce).

Data-parallel over nodes across 8 NeuronCores. Per core, per 512-node tile:
  - bf16 feature-major node features stream through TensorE matmuls
    (halves HBM traffic vs fp32; enables FWL fast weight loads)
  - ScalarE: silu directly + sigmoid-as-tanh (both in the silu_and_others
    ACT table -> one table load total); gate = 0.5*(1+tanh(x/2)) with the
    0.5 folded into the output weights
  - VectorE: one STT (gate product) + one batched TT (srcN product)
  - per-128-node-chunk segment reduction via one-hot matmul accumulating
    into a persistent PSUM (6, AW) window accumulator
Host side: shard + transpose + pack bf16 inputs, sum per-core (G,6)
partials, tiny (G,6)->(G,3,3) assembly in numpy.

All matmul PSUM outputs start at partition 0 (walrus rejects nonzero dst
base partitions); partition-packing via zero-extended stationary operands
+ PSUM accumulate. x2b's v=4 block uses 4 row-group-tiled K=32 matmuls
(tile_position inferred from matching lhsT/rhs base partitions).
"""
import math
import numpy as np
from contextlib import ExitStack

N_CORES = 8
CH = 128        # nodes per chunk (PE contraction dim)
TILE = 512      # nodes per compute tile (4 chunks)
AW = 512        # accumulator column window (graphs per core, padded)

# knobs (test.py may flip these)
TRACE = False
SIM_ACT = False   # True: substitute Tanh for Silu (CoreSim lacks Silu)
X2B_RG = False    # True: row-group tile_position matmuls for the v=4 block
X2B_K32 = False   # K=32 matmuls fault this HW (NRT_EXEC_UNIT_UNRECOVERABLE)
REPS = 1          # repeat the whole pipeline in-NEFF (timing); host divides
LAST_RESULTS = None

_cache = {}

# wconst column layout (bf16)
C_T1, C_H0X, C_GP, C_H2A, C_W2B4, C_W2B, C_SE, C_R6, WCOLS = \
    0, 64, 192, 272, 352, 368, 432, 444, 450
# blob per-tile column layout (bf16)
B_XS, B_X0, B_X2A, B_X2B, B_MC = 0, 512, 1024, 1536, 1664


def _build(NT, C, L, w0, reps=1, bias_sr_nz=False, bias6_nz=False,
           mc_off=B_MC):
    import concourse.tile as tile
    from concourse import bacc, mybir

    bf16 = mybir.dt.bfloat16
    f32 = mybir.dt.float32
    AF = mybir.ActivationFunctionType
    OP = mybir.AluOpType

    nc = bacc.Bacc("TRN2", target_bir_lowering=False, debug=False,
                   num_devices=N_CORES)
    blob = nc.dram_tensor("blob", [128, NT * C], bf16, kind="ExternalInput").ap()
    if X2B_K32:
        blobb = nc.dram_tensor("blobb", [32, NT * 512], bf16,
                               kind="ExternalInput").ap()
    wconst = nc.dram_tensor("wconst", [128, WCOLS], bf16, kind="ExternalInput").ap()
    bconst = nc.dram_tensor("bconst", [128, 16], f32, kind="ExternalInput").ap()
    out = nc.dram_tensor("out", [6, AW], f32, kind="ExternalOutput").ap()

    with tile.TileContext(nc) as tc, ExitStack() as ctx:
        cpool = ctx.enter_context(tc.tile_pool(name="consts", bufs=1))
        inpool = ctx.enter_context(tc.tile_pool(name="inp", bufs=8))
        mid = ctx.enter_context(tc.tile_pool(name="mid", bufs=3))
        opool = ctx.enter_context(tc.tile_pool(name="outp", bufs=1))
        psA = ctx.enter_context(tc.tile_pool(name="psA", bufs=2, space="PSUM"))
        psB = ctx.enter_context(tc.tile_pool(name="psB", bufs=1, space="PSUM"))
        psC = ctx.enter_context(tc.tile_pool(name="psC", bufs=2, space="PSUM"))
        psN = ctx.enter_context(tc.tile_pool(name="psN", bufs=2, space="PSUM"))
        psACC = ctx.enter_context(tc.tile_pool(name="psACC", bufs=1, space="PSUM"))

        wc = cpool.tile([128, WCOLS], bf16)
        bc = cpool.tile([128, 16], f32)
        nc.sync.dma_start(wc[:], wconst[:])
        nc.sync.dma_start(bc[:], bconst[:])
        acc = psACC.tile([6, AW], f32)
        nc.vector.memset(acc[:], 0.0)

        def _tile_body(t, bt, btb):
            # t1 pre-act rows 0:64, h0 pre-act rows 64:128 (zero-extended
            # lhsT + psum accumulate)
            pA = psA.tile([128, TILE], f32, tag="pA")
            nc.tensor.matmul(pA[0:128, :], wc[:, C_H0X:C_H0X + 128],
                             bt[:, B_X0:B_X0 + 512], start=True, stop=True)
            nc.tensor.matmul(pA[0:64, :], wc[:, C_T1:C_T1 + 64],
                             bt[:, B_XS:B_XS + 512], start=False, stop=True,
                             skip_group_check=True)
            # gate pre-act: rows 0:16 the 16 gate channels, rows 16:80 the
            # same gates replicated for v=0..3 (folded through W_e0)
            pB = psB.tile([80, TILE], f32, tag="pB")
            nc.tensor.matmul(pB[0:80, :], wc[:, C_GP:C_GP + 80],
                             bt[:, B_X0:B_X0 + 512], start=True, stop=True)
            # h2lin rows: 0:16 = v=4, 16:80 = v=0..3 block-diagonal
            pC = psC.tile([80, TILE], f32, tag="pC")
            nc.tensor.matmul(pC[0:80, :], wc[:, C_H2A:C_H2A + 80],
                             bt[:, B_X2A:B_X2A + 512], start=True, stop=True)
            if X2B_K32:
                nc.tensor.matmul(pC[0:16, :], wc[0:32, C_W2B4:C_W2B4 + 16],
                                 btb[:, :], start=False, stop=True,
                                 skip_group_check=True)
            for q in range(4 if not X2B_K32 else 0):
                if X2B_RG:
                    nc.tensor.matmul(pC[0:16, 128 * q:128 * (q + 1)],
                                     wc[32 * q:32 * (q + 1), C_W2B4:C_W2B4 + 16],
                                     bt[32 * q:32 * (q + 1), B_X2B:B_X2B + 128],
                                     start=False, stop=True,
                                     tile_position=(32 * q, 0),
                                     skip_group_check=True)
                else:
                    nc.tensor.matmul(pC[0:16, 128 * q:128 * (q + 1)],
                                     wc[:, C_W2B + 16 * q:C_W2B + 16 * (q + 1)],
                                     bt[:, B_X2B:B_X2B + 128],
                                     start=False, stop=True,
                                     skip_group_check=True)

            # ScalarE: silu directly; gate as tanh (same ACT table set).
            # gate = sigmoid(g) = 0.5*(1+tanh(g/2)); the 0.5 is folded into
            # R6 so h2g = (tanh+1)*pC.
            actT = mid.tile([128, TILE], bf16, tag="actT")
            nc.scalar.activation(actT[:], pA[:],
                                 AF.Tanh if SIM_ACT else AF.Silu,
                                 bias=bc[:, 0:1])
            tg = mid.tile([80, TILE], bf16, tag="tg")
            nc.scalar.activation(tg[:], pB[0:80, :], AF.Tanh,
                                 bias=bc[0:80, 1:2], scale=0.5)
            h2g = mid.tile([80, TILE], bf16, tag="h2g")
            nc.vector.scalar_tensor_tensor(h2g[:], tg[:], 1.0, pC[0:80, :],
                                           OP.add, OP.mult)

            # per-chunk tail: srep (cols 0:6) and e6 (cols 6:12) node-major
            # via activation-stationary matmuls; then one batched DVE mult
            # and per-chunk one-hot accumulation into the window accumulator
            pN = psN.tile([128, 48], f32, tag="pN")
            for c in range(4):
                o = 12 * c
                nc.tensor.matmul(pN[:, o:o + 12],
                                 actT[:, 128 * c:128 * (c + 1)],
                                 wc[:, C_SE:C_SE + 12], start=True, stop=True)
                nc.tensor.matmul(pN[:, o + 6:o + 12],
                                 h2g[:, 128 * c:128 * (c + 1)],
                                 wc[0:80, C_R6:C_R6 + 6], start=False, stop=True,
                                 skip_group_check=True)
            pN3 = pN[:].rearrange("p (c k) -> p c k", k=12)
            if bias_sr_nz:
                nc.vector.tensor_add(pN3[:, :, 0:6], pN3[:, :, 0:6],
                                     bc[:, 4:10].unsqueeze(1).broadcast_to([128, 4, 6]))
            if bias6_nz:
                nc.vector.tensor_add(pN3[:, :, 6:12], pN3[:, :, 6:12],
                                     bc[:, 10:16].unsqueeze(1).broadcast_to([128, 4, 6]))
            # DVE can read only one PSUM operand: stage the srep half in SBUF
            # via one batched ScalarE copy, then one batched DVE multiply
            srcS = mid.tile([128, 24], bf16, tag="srcS")
            nc.scalar.activation(srcS[:].rearrange("p (c k) -> p c k", k=6),
                                 pN3[:, :, 0:6], AF.Copy)
            srcN = mid.tile([128, 24], bf16, tag="srcN")
            nc.vector.tensor_tensor(
                srcN[:].rearrange("p (c k) -> p c k", k=6),
                srcS[:].rearrange("p (c k) -> p c k", k=6),
                pN3[:, :, 6:12], OP.mult)
            for c in range(4):
                cg = 4 * t + c
                nc.tensor.matmul(acc[0:6, w0[cg]:w0[cg] + L],
                                 srcN[:, 6 * c:6 * (c + 1)],
                                 bt[:, mc_off + L * c:mc_off + L * (c + 1)],
                                 start=False, stop=True,
                                 skip_group_check=True)

        def _tile_loop():
            for t in range(NT):
                bt = inpool.tile([128, C], bf16, tag="bt")
                # first two tiles ride the scalar HWDGE ring so they don't
                # queue behind wconst/bconst on the sync ring at cold start
                eng = nc.scalar if t < 2 else nc.sync
                eng.dma_start(bt[:], blob[:, t * C:(t + 1) * C])
                _tile_body(t, bt, None)

        if reps == 1:
            _tile_loop()
        else:
            with tc.For_i(0, reps, 1):
                _tile_loop()

        accS = opool.tile([6, AW], f32)
        nc.vector.tensor_copy(accS[:], acc[:])
        nc.sync.dma_start(out[:], accS[:])

    nc.compile()
    return nc


def kernel(node_scalar, node_equi, batch, n_graphs, W_s1, b_s1, W_s2, b_s2,
           W_e0, b_e0, W_e2, W_g, b_g, W_o0, b_o0, W_o2):
    global LAST_RESULTS
    import ml_dtypes
    from concourse.bass_utils import run_bass_kernel_spmd
    bf16 = ml_dtypes.bfloat16

    node_scalar = np.asarray(node_scalar, dtype=np.float32)
    node_equi = np.asarray(node_equi, dtype=np.float32)
    batch = np.asarray(batch).astype(np.int64)
    G = int(n_graphs)
    W_s1 = np.asarray(W_s1, np.float32); b_s1 = np.asarray(b_s1, np.float32)
    W_s2 = np.asarray(W_s2, np.float32); b_s2 = np.asarray(b_s2, np.float32)
    W_e0 = np.asarray(W_e0, np.float32); b_e0 = np.asarray(b_e0, np.float32)
    W_e2 = np.asarray(W_e2, np.float32)
    W_g = np.asarray(W_g, np.float32); b_g = np.asarray(b_g, np.float32)
    W_o0 = np.asarray(W_o0, np.float32); b_o0 = np.asarray(b_o0, np.float32)
    W_o2 = np.asarray(W_o2, np.float32)

    N = node_scalar.shape[0]
    assert N % N_CORES == 0, N
    PER = N // N_CORES
    NCH = (PER + CH - 1) // CH          # chunks per core
    NT = (NCH + 3) // 4                 # tiles per core
    NPAD = NT * TILE                    # padded nodes per core
    L = 16
    mc_off = B_X2B if X2B_K32 else B_MC
    C = mc_off + 4 * L

    # --- segment window planning (shared program, per-core data) ---
    g_first = np.array([batch[k * PER] for k in range(N_CORES)], np.int64)
    w0 = np.zeros(NCH, np.int64)
    need = 0
    for cg in range(NCH):
        lo = min(int(batch[k * PER + cg * CH]) - int(g_first[k])
                 for k in range(N_CORES))
        hi = max(int(batch[min(k * PER + (cg + 1) * CH, (k + 1) * PER) - 1])
                 - int(g_first[k]) for k in range(N_CORES))
        lo &= ~1          # matmul dst col offset kept even
        w0[cg] = lo
        need = max(need, hi - lo + 1)
    assert need <= L, f"window overflow: need {need} > L={L}"
    assert int((w0 + L).max()) <= AW, "acc width overflow"

    # --- weight folding (host) ---
    Wg16 = W_e0 @ W_g                                  # (128, 16)
    GPw = np.concatenate([Wg16, np.tile(Wg16, (1, 4))], axis=1)  # (128, 80)
    bg16 = W_g.T @ b_e0 + b_g                          # (16,)
    bg80 = np.concatenate([bg16, np.tile(bg16, 4)])    # (80,)
    # h2lin: rows 0:16 = v4 (row-group K=32 matmuls), rows 16:80 = v<4 diag
    W2A = np.zeros((128, 80), np.float32)
    for v in range(4):
        W2A[v * 32:(v + 1) * 32, 16 + v * 16:16 + (v + 1) * 16] = W_e2
    W2B4 = np.zeros((128, 16), np.float32)
    if X2B_K32:
        W2B4[0:32, :] = W_e2
    else:
        for q in range(4):
            W2B4[q * 32:(q + 1) * 32, :] = W_e2
    W2B = np.zeros((128, 64), np.float32)
    for q in range(4):
        W2B[q * 32:(q + 1) * 32, q * 16:(q + 1) * 16] = W_e2
    Q = np.zeros((2, 6), np.float32)
    Q[0, 0] = 1.0
    Q[1, 1:6] = 1.0
    WsQ = W_s2 @ Q                                     # (64, 6)
    bias_sr = Q.T @ b_s2                               # (6,)
    W6A = np.zeros((64, 6), np.float32)
    W6A[:, 0] = W_o0[:, 0]
    SEw = np.zeros((128, 12), np.float32)
    SEw[0:64, 0:6] = WsQ
    SEw[64:128, 6:12] = W6A
    # 0.5 from gate=0.5*(1+tanh) folded here
    R6 = np.zeros((80, 6), np.float32)
    R6[0:16, 5] = 0.5 * W_o2[:, 0]                     # v=4 block
    for v in range(4):
        R6[16 + v * 16:16 + (v + 1) * 16, 1 + v] = 0.5 * W_o2[:, 0]
    bias6 = np.zeros(6, np.float32)
    bias6[0] = b_o0[0]

    wconst = np.zeros((128, WCOLS), np.float32)
    wconst[:, C_T1:C_T1 + 64] = W_s1
    wconst[:, C_H0X + 64:C_H0X + 128] = W_e0           # cols 0:64 stay zero
    wconst[:, C_GP:C_GP + 80] = GPw                    # cols 80:128 zero
    wconst[:, C_H2A:C_H2A + 80] = W2A                  # cols 0:16, 80:128 zero
    wconst[:, C_W2B4:C_W2B4 + 16] = W2B4
    wconst[:, C_W2B:C_W2B + 64] = W2B
    wconst[:, C_SE:C_SE + 12] = SEw
    wconst[0:80, C_R6:C_R6 + 6] = R6
    wconst = wconst.astype(bf16)

    bconst = np.zeros((128, 16), np.float32)
    bconst[0:64, 0] = b_s1
    bconst[64:128, 0] = b_e0
    bconst[0:80, 1] = 0.5 * bg80                       # tanh(0.5*(g+bg))
    bconst[:, 4:10] = bias_sr[None, :]
    bconst[:, 10:16] = bias6[None, :]

    # --- per-core blobs ---
    in_maps = []
    for k in range(N_CORES):
        s0, s1 = k * PER, (k + 1) * PER
        xs = np.zeros((128, NPAD), np.float32)
        xs[:, :PER] = node_scalar[s0:s1].T
        x0 = np.zeros((128, NPAD), np.float32)
        x0[:, :PER] = node_equi[s0:s1, 0:128].T
        e2 = node_equi[s0:s1, 320:480].reshape(PER, 32, 5).transpose(2, 1, 0)
        x2a = np.zeros((128, NPAD), np.float32)
        x2a[:, :PER] = e2[:4].reshape(128, PER)
        x2b = np.zeros((32, NPAD), np.float32)
        x2b[:, :PER] = e2[4]
        # pack x2b (32, NT*512) -> (128, NT, 128): partition 32q+m holds
        # nodes 512t+128q+j at column j
        x2bp = x2b.reshape(32, NT, 4, 128).transpose(2, 0, 1, 3).reshape(128, NT, 128)
        # one-hot segment maps, relative to per-chunk windows
        rel = np.full(NPAD, -1, np.int64)
        bloc = batch[s0:s1] - g_first[k]
        for cg in range(NCH):
            a, b = cg * CH, min((cg + 1) * CH, PER)
            rel[a:b] = bloc[a:b] - w0[cg]
        mc = (rel[:, None] == np.arange(L)[None, :]).astype(np.float32)
        mcp = mc.reshape(NT, 4, 128, L).transpose(2, 0, 1, 3).reshape(128, NT, 4 * L)

        blob = np.empty((128, NT, C), np.float32)
        blob[:, :, B_XS:B_XS + 512] = xs.reshape(128, NT, 512)
        blob[:, :, B_X0:B_X0 + 512] = x0.reshape(128, NT, 512)
        blob[:, :, B_X2A:B_X2A + 512] = x2a.reshape(128, NT, 512)
        if not X2B_K32:
            blob[:, :, B_X2B:B_X2B + 128] = x2bp
        blob[:, :, mc_off:mc_off + 4 * L] = mcp
        blob = blob.reshape(128, NT * C).astype(bf16)
        im = {"blob": blob, "wconst": wconst, "bconst": bconst}
        if X2B_K32:
            im["blobb"] = x2b.astype(bf16)
        in_maps.append(im)

    # --- build (cached) + run ---
    bias_sr_nz = bool(np.any(bias_sr != 0))
    bias6_nz = bool(np.any(bias6 != 0))
    key = (N, G, NT, C, L, REPS, X2B_RG, X2B_K32, bias_sr_nz, bias6_nz, tuple(w0.tolist()))
    if key not in _cache:
        _cache[key] = _build(NT, C, L, w0, REPS, bias_sr_nz, bias6_nz,
                             mc_off=mc_off)
    nc = _cache[key]

    res = run_bass_kernel_spmd(nc, in_maps, list(range(N_CORES)), trace=TRACE)
    LAST_RESULTS = res

    # --- host unshard: sum windowed partials, assemble 3x3 ---
    polar6 = np.zeros((G + AW, 6), np.float64)
    for k in range(N_CORES):
        o = res.results[k]["out"]                      # (6, AW)
        polar6[g_first[k]:g_first[k] + AW] += o.T.astype(np.float64) / REPS
    polar6 = polar6[:G]

    zero = polar6[:, 0]
    d = polar6[:, 1:6]
    d_norm = np.sqrt((d * d).sum(-1))
    dxy, dyz, dz2, dzx, dx2y2 = d[:, 0], d[:, 1], d[:, 2], d[:, 3], d[:, 4]
    cc = 1.0 / math.sqrt(3.0)
    a00 = zero + cc * (d_norm - dz2) + dx2y2
    a11 = zero + cc * (d_norm - dz2) - dx2y2
    a22 = zero + cc * (d_norm + 2.0 * dz2)
    outm = np.empty((G, 3, 3), np.float64)
    outm[:, 0, 0] = a00; outm[:, 0, 1] = dxy; outm[:, 0, 2] = dzx
    outm[:, 1, 0] = dxy; outm[:, 1, 1] = a11; outm[:, 1, 2] = dyz
    outm[:, 2, 0] = dzx; outm[:, 2, 1] = dyz; outm[:, 2, 2] = a22
    return outm.astype(np.float32)


# revision 26
# speedup vs baseline: 1.1847x; 1.1847x over previous
"""Trainium2 Bass kernel for nn_PolarOut (segment_reduce).

Data-parallel over nodes across 8 NeuronCores. Per core, per 512-node tile:
  - bf16 feature-major node features stream through TensorE matmuls
    (halves HBM traffic vs fp32; enables FWL fast weight loads)
  - ScalarE: silu directly + sigmoid-as-tanh (both in the silu_and_others
    ACT table -> one table load total); gate = 0.5*(1+tanh(x/2)) with the
    0.5 folded into the output weights
  - VectorE: one STT (gate product) + one batched TT (srcN product)
  - per-128-node-chunk segment reduction via one-hot matmul accumulating
    into a persistent PSUM (6, AW) window accumulator
Host side: shard + transpose + pack bf16 inputs, sum per-core (G,6)
partials, tiny (G,6)->(G,3,3) assembly in numpy.

All matmul PSUM outputs start at partition 0 (walrus rejects nonzero dst
base partitions); partition-packing via zero-extended stationary operands
+ PSUM accumulate. x2b's v=4 block uses 4 row-group-tiled K=32 matmuls
(tile_position inferred from matching lhsT/rhs base partitions).
"""
import math
import numpy as np
from contextlib import ExitStack

N_CORES = 8
CH = 128        # nodes per chunk (PE contraction dim)
TILE = 512      # nodes per compute tile (4 chunks)
AW = 512        # accumulator column window (graphs per core, padded)

# knobs (test.py may flip these)
TRACE = False
SIM_ACT = False   # True: substitute Tanh for Silu (CoreSim lacks Silu)
X2B_RG = False    # True: row-group tile_position matmuls for the v=4 block
X2B_K32 = False   # K=32 matmuls fault this HW (NRT_EXEC_UNIT_UNRECOVERABLE)
REPS = 1          # repeat the whole pipeline in-NEFF (timing); host divides
LAST_RESULTS = None

_cache = {}

# wconst column layout (bf16)
C_T1, C_H0X, C_GP, C_H2A, C_W2B4, C_W2B, C_SE, C_R6, WCOLS = \
    0, 64, 192, 272, 352, 368, 432, 444, 450
# blob per-tile column layout (bf16)
B_XS, B_X0, B_X2A, B_X2B, B_MC = 0, 512, 1024, 1536, 1664


def _build(NT, C, L, w0, reps=1, bias_sr_nz=False, bias6_nz=False,
           mc_off=B_MC):
    import concourse.tile as tile
    from concourse import bacc, mybir

    bf16 = mybir.dt.bfloat16
    f32 = mybir.dt.float32
    AF = mybir.ActivationFunctionType
    OP = mybir.AluOpType

    nc = bacc.Bacc("TRN2", target_bir_lowering=False, debug=False,
                   num_devices=N_CORES)
    blob = nc.dram_tensor("blob", [128, NT * C], bf16, kind="ExternalInput").ap()
    if X2B_K32:
        blobb = nc.dram_tensor("blobb", [32, NT * 512], bf16,
                               kind="ExternalInput").ap()
    wconst = nc.dram_tensor("wconst", [128, WCOLS], bf16, kind="ExternalInput").ap()
    bconst = nc.dram_tensor("bconst", [128, 16], f32, kind="ExternalInput").ap()
    out = nc.dram_tensor("out", [6, AW], f32, kind="ExternalOutput").ap()

    with tile.TileContext(nc) as tc, ExitStack() as ctx:
        cpool = ctx.enter_context(tc.tile_pool(name="consts", bufs=1))
        inpool = ctx.enter_context(tc.tile_pool(name="inp", bufs=10))
        mid = ctx.enter_context(tc.tile_pool(name="mid", bufs=4))
        opool = ctx.enter_context(tc.tile_pool(name="outp", bufs=1))
        psA = ctx.enter_context(tc.tile_pool(name="psA", bufs=2, space="PSUM"))
        psB = ctx.enter_context(tc.tile_pool(name="psB", bufs=1, space="PSUM"))
        psC = ctx.enter_context(tc.tile_pool(name="psC", bufs=2, space="PSUM"))
        psN = ctx.enter_context(tc.tile_pool(name="psN", bufs=2, space="PSUM"))
        psACC = ctx.enter_context(tc.tile_pool(name="psACC", bufs=1, space="PSUM"))

        wc = cpool.tile([128, WCOLS], bf16)
        bc = cpool.tile([128, 16], f32)
        nc.sync.dma_start(wc[:], wconst[:])
        nc.sync.dma_start(bc[:], bconst[:])
        acc = psACC.tile([6, AW], f32)
        nc.vector.memset(acc[:], 0.0)

        def _tile_body(t, bt, btb):
            # t1 pre-act rows 0:64, h0 pre-act rows 64:128 (zero-extended
            # lhsT + psum accumulate)
            pA = psA.tile([128, TILE], f32, tag="pA")
            nc.tensor.matmul(pA[0:128, :], wc[:, C_H0X:C_H0X + 128],
                             bt[:, B_X0:B_X0 + 512], start=True, stop=True)
            nc.tensor.matmul(pA[0:64, :], wc[:, C_T1:C_T1 + 64],
                             bt[:, B_XS:B_XS + 512], start=False, stop=True,
                             skip_group_check=True)
            # gate pre-act: rows 0:16 the 16 gate channels, rows 16:80 the
            # same gates replicated for v=0..3 (folded through W_e0)
            pB = psB.tile([80, TILE], f32, tag="pB")
            nc.tensor.matmul(pB[0:80, :], wc[:, C_GP:C_GP + 80],
                             bt[:, B_X0:B_X0 + 512], start=True, stop=True)
            # h2lin rows: 0:16 = v=4, 16:80 = v=0..3 block-diagonal
            pC = psC.tile([80, TILE], f32, tag="pC")
            nc.tensor.matmul(pC[0:80, :], wc[:, C_H2A:C_H2A + 80],
                             bt[:, B_X2A:B_X2A + 512], start=True, stop=True)
            if X2B_K32:
                nc.tensor.matmul(pC[0:16, :], wc[0:32, C_W2B4:C_W2B4 + 16],
                                 btb[:, :], start=False, stop=True,
                                 skip_group_check=True)
            for q in range(4 if not X2B_K32 else 0):
                if X2B_RG:
                    nc.tensor.matmul(pC[0:16, 128 * q:128 * (q + 1)],
                                     wc[32 * q:32 * (q + 1), C_W2B4:C_W2B4 + 16],
                                     bt[32 * q:32 * (q + 1), B_X2B:B_X2B + 128],
                                     start=False, stop=True,
                                     tile_position=(32 * q, 0),
                                     skip_group_check=True)
                else:
                    nc.tensor.matmul(pC[0:16, 128 * q:128 * (q + 1)],
                                     wc[:, C_W2B + 16 * q:C_W2B + 16 * (q + 1)],
                                     bt[:, B_X2B:B_X2B + 128],
                                     start=False, stop=True,
                                     skip_group_check=True)

            # ScalarE: silu directly; gate as tanh (same ACT table set).
            # gate = sigmoid(g) = 0.5*(1+tanh(g/2)); the 0.5 is folded into
            # R6 so h2g = (tanh+1)*pC.
            actT = mid.tile([128, TILE], bf16, tag="actT")
            nc.scalar.activation(actT[:], pA[:],
                                 AF.Tanh if SIM_ACT else AF.Silu,
                                 bias=bc[:, 0:1])
            tg = mid.tile([80, TILE], bf16, tag="tg")
            nc.scalar.activation(tg[:], pB[0:80, :], AF.Tanh,
                                 bias=bc[0:80, 1:2], scale=0.5)
            h2g = mid.tile([80, TILE], bf16, tag="h2g")
            nc.vector.scalar_tensor_tensor(h2g[:], tg[:], 1.0, pC[0:80, :],
                                           OP.add, OP.mult)

            # per-chunk tail: srep (cols 0:6) and e6 (cols 6:12) node-major
            # via activation-stationary matmuls; then one batched DVE mult
            # and per-chunk one-hot accumulation into the window accumulator
            pN = psN.tile([128, 48], f32, tag="pN")
            for c in range(4):
                o = 12 * c
                nc.tensor.matmul(pN[:, o:o + 12],
                                 actT[:, 128 * c:128 * (c + 1)],
                                 wc[:, C_SE:C_SE + 12], start=True, stop=True)
                nc.tensor.matmul(pN[:, o + 6:o + 12],
                                 h2g[:, 128 * c:128 * (c + 1)],
                                 wc[0:80, C_R6:C_R6 + 6], start=False, stop=True,
                                 skip_group_check=True)
            pN3 = pN[:].rearrange("p (c k) -> p c k", k=12)
            if bias_sr_nz:
                nc.vector.tensor_add(pN3[:, :, 0:6], pN3[:, :, 0:6],
                                     bc[:, 4:10].unsqueeze(1).broadcast_to([128, 4, 6]))
            if bias6_nz:
                nc.vector.tensor_add(pN3[:, :, 6:12], pN3[:, :, 6:12],
                                     bc[:, 10:16].unsqueeze(1).broadcast_to([128, 4, 6]))
            # DVE can read only one PSUM operand: stage the srep half in SBUF
            # via one batched ScalarE copy, then one batched DVE multiply
            srcS = mid.tile([128, 24], bf16, tag="srcS")
            nc.scalar.activation(srcS[:].rearrange("p (c k) -> p c k", k=6),
                                 pN3[:, :, 0:6], AF.Copy)
            srcN = mid.tile([128, 24], bf16, tag="srcN")
            nc.vector.tensor_tensor(
                srcN[:].rearrange("p (c k) -> p c k", k=6),
                srcS[:].rearrange("p (c k) -> p c k", k=6),
                pN3[:, :, 6:12], OP.mult)
            for c in range(4):
                cg = 4 * t + c
                nc.tensor.matmul(acc[0:6, w0[cg]:w0[cg] + L],
                                 srcN[:, 6 * c:6 * (c + 1)],
                                 bt[:, mc_off + L * c:mc_off + L * (c + 1)],
                                 start=False, stop=True,
                                 skip_group_check=True)

        def _tile_loop():
            for t in range(NT):
                bt = inpool.tile([128, C], bf16, tag="bt")
                nc.sync.dma_start(bt[:], blob[:, t * C:(t + 1) * C])
                _tile_body(t, bt, None)

        if reps == 1:
            _tile_loop()
        else:
            with tc.For_i(0, reps, 1):
                _tile_loop()

        accS = opool.tile([6, AW], f32)
        nc.vector.tensor_copy(accS[:], acc[:])
        nc.sync.dma_start(out[:], accS[:])

    nc.compile()
    return nc


def kernel(node_scalar, node_equi, batch, n_graphs, W_s1, b_s1, W_s2, b_s2,
           W_e0, b_e0, W_e2, W_g, b_g, W_o0, b_o0, W_o2):
    global LAST_RESULTS
    import ml_dtypes
    from concourse.bass_utils import run_bass_kernel_spmd
    bf16 = ml_dtypes.bfloat16

    node_scalar = np.asarray(node_scalar, dtype=np.float32)
    node_equi = np.asarray(node_equi, dtype=np.float32)
    batch = np.asarray(batch).astype(np.int64)
    G = int(n_graphs)
    W_s1 = np.asarray(W_s1, np.float32); b_s1 = np.asarray(b_s1, np.float32)
    W_s2 = np.asarray(W_s2, np.float32); b_s2 = np.asarray(b_s2, np.float32)
    W_e0 = np.asarray(W_e0, np.float32); b_e0 = np.asarray(b_e0, np.float32)
    W_e2 = np.asarray(W_e2, np.float32)
    W_g = np.asarray(W_g, np.float32); b_g = np.asarray(b_g, np.float32)
    W_o0 = np.asarray(W_o0, np.float32); b_o0 = np.asarray(b_o0, np.float32)
    W_o2 = np.asarray(W_o2, np.float32)

    N = node_scalar.shape[0]
    assert N % N_CORES == 0, N
    PER = N // N_CORES
    NCH = (PER + CH - 1) // CH          # chunks per core
    NT = (NCH + 3) // 4                 # tiles per core
    NPAD = NT * TILE                    # padded nodes per core
    mc_off = B_X2B if X2B_K32 else B_MC

    # --- segment window planning (shared program, per-core data) ---
    g_first = np.array([batch[k * PER] for k in range(N_CORES)], np.int64)
    w0 = np.zeros(NCH, np.int64)
    need = 0
    for cg in range(NCH):
        lo = min(int(batch[k * PER + cg * CH]) - int(g_first[k])
                 for k in range(N_CORES))
        hi = max(int(batch[min(k * PER + (cg + 1) * CH, (k + 1) * PER) - 1])
                 - int(g_first[k]) for k in range(N_CORES))
        lo &= ~1          # matmul dst col offset kept even
        w0[cg] = lo
        need = max(need, hi - lo + 1)
    L = max(16, (need + 1) & ~1)   # one-hot window width (even)
    C = mc_off + 4 * L
    assert int((w0 + L).max()) <= AW, "acc width overflow"

    # --- weight folding (host) ---
    Wg16 = W_e0 @ W_g                                  # (128, 16)
    GPw = np.concatenate([Wg16, np.tile(Wg16, (1, 4))], axis=1)  # (128, 80)
    bg16 = W_g.T @ b_e0 + b_g                          # (16,)
    bg80 = np.concatenate([bg16, np.tile(bg16, 4)])    # (80,)
    # h2lin: rows 0:16 = v4 (row-group K=32 matmuls), rows 16:80 = v<4 diag
    W2A = np.zeros((128, 80), np.float32)
    for v in range(4):
        W2A[v * 32:(v + 1) * 32, 16 + v * 16:16 + (v + 1) * 16] = W_e2
    W2B4 = np.zeros((128, 16), np.float32)
    if X2B_K32:
        W2B4[0:32, :] = W_e2
    else:
        for q in range(4):
            W2B4[q * 32:(q + 1) * 32, :] = W_e2
    W2B = np.zeros((128, 64), np.float32)
    for q in range(4):
        W2B[q * 32:(q + 1) * 32, q * 16:(q + 1) * 16] = W_e2
    Q = np.zeros((2, 6), np.float32)
    Q[0, 0] = 1.0
    Q[1, 1:6] = 1.0
    WsQ = W_s2 @ Q                                     # (64, 6)
    bias_sr = Q.T @ b_s2                               # (6,)
    W6A = np.zeros((64, 6), np.float32)
    W6A[:, 0] = W_o0[:, 0]
    SEw = np.zeros((128, 12), np.float32)
    SEw[0:64, 0:6] = WsQ
    SEw[64:128, 6:12] = W6A
    # 0.5 from gate=0.5*(1+tanh) folded here
    R6 = np.zeros((80, 6), np.float32)
    R6[0:16, 5] = 0.5 * W_o2[:, 0]                     # v=4 block
    for v in range(4):
        R6[16 + v * 16:16 + (v + 1) * 16, 1 + v] = 0.5 * W_o2[:, 0]
    bias6 = np.zeros(6, np.float32)
    bias6[0] = b_o0[0]

    wconst = np.zeros((128, WCOLS), np.float32)
    wconst[:, C_T1:C_T1 + 64] = W_s1
    wconst[:, C_H0X + 64:C_H0X + 128] = W_e0           # cols 0:64 stay zero
    wconst[:, C_GP:C_GP + 80] = GPw                    # cols 80:128 zero
    wconst[:, C_H2A:C_H2A + 80] = W2A                  # cols 0:16, 80:128 zero
    wconst[:, C_W2B4:C_W2B4 + 16] = W2B4
    wconst[:, C_W2B:C_W2B + 64] = W2B
    wconst[:, C_SE:C_SE + 12] = SEw
    wconst[0:80, C_R6:C_R6 + 6] = R6
    wconst = wconst.astype(bf16)

    bconst = np.zeros((128, 16), np.float32)
    bconst[0:64, 0] = b_s1
    bconst[64:128, 0] = b_e0
    bconst[0:80, 1] = 0.5 * bg80                       # tanh(0.5*(g+bg))
    bconst[:, 4:10] = bias_sr[None, :]
    bconst[:, 10:16] = bias6[None, :]

    # --- per-core blobs ---
    in_maps = []
    for k in range(N_CORES):
        s0, s1 = k * PER, (k + 1) * PER
        xs = np.zeros((128, NPAD), np.float32)
        xs[:, :PER] = node_scalar[s0:s1].T
        x0 = np.zeros((128, NPAD), np.float32)
        x0[:, :PER] = node_equi[s0:s1, 0:128].T
        e2 = node_equi[s0:s1, 320:480].reshape(PER, 32, 5).transpose(2, 1, 0)
        x2a = np.zeros((128, NPAD), np.float32)
        x2a[:, :PER] = e2[:4].reshape(128, PER)
        x2b = np.zeros((32, NPAD), np.float32)
        x2b[:, :PER] = e2[4]
        # pack x2b (32, NT*512) -> (128, NT, 128): partition 32q+m holds
        # nodes 512t+128q+j at column j
        x2bp = x2b.reshape(32, NT, 4, 128).transpose(2, 0, 1, 3).reshape(128, NT, 128)
        # one-hot segment maps, relative to per-chunk windows
        rel = np.full(NPAD, -1, np.int64)
        bloc = batch[s0:s1] - g_first[k]
        for cg in range(NCH):
            a, b = cg * CH, min((cg + 1) * CH, PER)
            rel[a:b] = bloc[a:b] - w0[cg]
        mc = (rel[:, None] == np.arange(L)[None, :]).astype(np.float32)
        mcp = mc.reshape(NT, 4, 128, L).transpose(2, 0, 1, 3).reshape(128, NT, 4 * L)

        blob = np.empty((128, NT, C), np.float32)
        blob[:, :, B_XS:B_XS + 512] = xs.reshape(128, NT, 512)
        blob[:, :, B_X0:B_X0 + 512] = x0.reshape(128, NT, 512)
        blob[:, :, B_X2A:B_X2A + 512] = x2a.reshape(128, NT, 512)
        if not X2B_K32:
            blob[:, :, B_X2B:B_X2B + 128] = x2bp
        blob[:, :, mc_off:mc_off + 4 * L] = mcp
        blob = blob.reshape(128, NT * C).astype(bf16)
        im = {"blob": blob, "wconst": wconst, "bconst": bconst}
        if X2B_K32:
            im["blobb"] = x2b.astype(bf16)
        in_maps.append(im)

    # --- build (cached) + run ---
    bias_sr_nz = bool(np.any(bias_sr != 0))
    bias6_nz = bool(np.any(bias6 != 0))
    key = (N, G, NT, C, L, REPS, X2B_RG, X2B_K32, bias_sr_nz, bias6_nz, tuple(w0.tolist()))
    if key not in _cache:
        _cache[key] = _build(NT, C, L, w0, REPS, bias_sr_nz, bias6_nz,
                             mc_off=mc_off)
    nc = _cache[key]

    res = run_bass_kernel_spmd(nc, in_maps, list(range(N_CORES)), trace=TRACE)
    LAST_RESULTS = res

    # --- host unshard: sum windowed partials, assemble 3x3 ---
    polar6 = np.zeros((G + AW, 6), np.float64)
    for k in range(N_CORES):
        o = res.results[k]["out"]                      # (6, AW)
        polar6[g_first[k]:g_first[k] + AW] += o.T.astype(np.float64) / REPS
    polar6 = polar6[:G]

    zero = polar6[:, 0]
    d = polar6[:, 1:6]
    d_norm = np.sqrt((d * d).sum(-1))
    dxy, dyz, dz2, dzx, dx2y2 = d[:, 0], d[:, 1], d[:, 2], d[:, 3], d[:, 4]
    cc = 1.0 / math.sqrt(3.0)
    a00 = zero + cc * (d_norm - dz2) + dx2y2
    a11 = zero + cc * (d_norm - dz2) - dx2y2
    a22 = zero + cc * (d_norm + 2.0 * dz2)
    outm = np.empty((G, 3, 3), np.float64)
    outm[:, 0, 0] = a00; outm[:, 0, 1] = dxy; outm[:, 0, 2] = dzx
    outm[:, 1, 0] = dxy; outm[:, 1, 1] = a11; outm[:, 1, 2] = dyz
    outm[:, 2, 0] = dzx; outm[:, 2, 1] = dyz; outm[:, 2, 2] = a22
    return outm.astype(np.float32)
